# revision 29
# baseline (speedup 1.0000x reference)
"""Trainium2 Bass kernel for a 3-layer GRU (T=2048, B=64, IN=H=256).

Strategy (8 NeuronCores, data-parallel over batch, B_local=8 per core):
  - Everything lives H-major on-chip: hidden vectors are [H(part), batch(free)].
  - Per layer a PSUM window of W=8 timesteps holds the gate pre-activations:
      RZ  = x@Wih_rz + b_ih_rz + b_hh_rz  (+= Whh_rz @ h per step)
      HN  = b_hh_n                        (+= Whh_n  @ h per step)
      XN  = x@Wih_n  + b_ih_n             (bulk only)
    The bulk input projection is a real matmul accumulated straight into
    PSUM (start=True bias matmul with a ones-vector rhs seeds the biases).
  - The sequential scan runs layer-pipelined: at tick t, layer0 processes
    step t, layer1 step t-8, layer2 step t-16. Gate elementwise ops for all
    active layers are packed into single wide instructions ([128, 48]-ish).
  - h state lives in a 32-slot SBUF ring [128, 48*32]; slot = tick % 32.
  - Recurrent matmuls are weight-stationary, column-tiled 4x (tile_position)
    so four 32-col LDWEIGHTS/MATMULs proceed concurrently.
  - Steady state runs under tc.For_i with a 4-group (32-tick) body so all
    ring/PSUM offsets are static; only DRAM DMA offsets use the loop var.
"""

import os
import numpy as np

try:
    import concourse.bass as bass
except ImportError:  # pragma: no cover
    import sys

    for p in ("/opt/trn_rl_repo", "/root/.axon_site/_ro/trn_rl_repo"):
        if os.path.isdir(p) and p not in sys.path:
            sys.path.insert(0, p)
    import concourse.bass as bass

import concourse.mybir as mybir
import concourse.tile as tile
from concourse.bass_utils import run_bass_kernel_spmd

FP = mybir.dt.float32
AF = mybir.ActivationFunctionType
ALU = mybir.AluOpType

N_CORES = 8
T_FULL = 2048
B = 8  # batch per core
H = 256
G3 = 768
L = 3
W = 8  # steps per window (= chunk)
RING = 32  # ring slots (ticks); multiple of W, >= 4 windows
GRP_PER_ITER = 4  # groups per For_i iteration (RING // W)
LANES = L * 2 * B  # 48 cols per ring slot: (layer, h-chunk, batch)

# consts blob column offsets: [wih | whh | eye | bias8 | sel8 | h0]
OFF_WIH = 0
OFF_WHH = 6 * G3
OFF_EYE = 12 * G3
OFF_B8 = 12 * G3 + 128
OFF_SEL = OFF_B8 + L * 128
OFF_H0 = OFF_SEL + 512
CONST_COLS = OFF_H0 + L * H

# Knobs
U_ON_GPSIMD = False  # z*h_prev on gpsimd instead of vector
COL_TILE_RECUR = False  # 4x column-tiled recurrent matmuls

LAST_EXEC_NS = None
LAST_RESULTS = None


def _subreg(m):
    # Wih/Whh output M-tile m (gate rows 128m..128m+128) -> PSUM subregion.
    # Subregions: 0..3 = RZ, 4..5 = HN (recurrent n), 6..7 = XN (bulk n).
    return m if m < 4 else m + 2


def build_program(T, legalize=True):
    assert T % (W * GRP_PER_ITER) == 0
    n_chunks = T // W
    n_groups = n_chunks + L - 1

    nc = bass.Bass()

    x_d = nc.declare_dram_parameter("x", [T * B, H], FP, isOutput=False)
    consts_d = nc.declare_dram_parameter("consts", [128, CONST_COLS], FP,
                                         isOutput=False)
    out_d = nc.declare_dram_parameter("out", [T * B, H], FP, isOutput=True)
    hn_d = nc.declare_dram_parameter("h_n", [B, L * H], FP, isOutput=True)
    dbg_d = None
    if os.environ.get("GRU_DEBUG_RING"):
        dbg_d = nc.declare_dram_parameter("dbg", [128, LANES * RING + 7 * 512], FP,
                                          isOutput=True)
    globals()["_DBG_D"] = dbg_d

    with tile.TileContext(nc) as tc:
        _build_tiles(tc, nc, T, n_chunks, n_groups,
                     x_d, consts_d, out_d, hn_d)
    if legalize:
        _legalize_waits(nc)
    return nc


def _legalize_waits(nc):
    """This walrus build allows only ONE embedded sync-wait per compute
    instruction (LDWEIGHTS/ACT structs overflow at 2). Hoist extra waits
    onto an inserted same-engine Drain, which supports many waits."""
    CAP_DRAIN = 1
    n_fix = 0
    for f in nc.m.functions:
        for b in f.blocks:
            out = []
            changed = False
            for inst in b.instructions:
                si = getattr(inst, 'sync_info', None)
                waits = list(si.on_wait) if si is not None and si.on_wait else []
                opname = type(inst).__name__
                cap = CAP_DRAIN if opname == "InstDrain" else 1
                if len(waits) > cap:
                    extra = waits[:-cap]
                    si.on_wait = waits[-cap:]
                    for j in range(0, len(extra), CAP_DRAIN):
                        d = mybir.InstDrain(name=f"I-wfix-{n_fix}", ins=[], outs=[])
                        n_fix += 1
                        d.engine = inst.engine
                        d.sync_info = mybir.SyncInfo(
                            on_wait=extra[j:j + CAP_DRAIN], on_update=[])
                        out.append(d)
                    changed = True
                out.append(inst)
            if changed:
                b.instructions = out
    return n_fix


def _build_tiles(tc, nc, T, n_chunks, n_groups,
                 x_d, consts_d, out_d, hn_d):
    from contextlib import ExitStack

    ctx = ExitStack()
    const = ctx.enter_context(tc.tile_pool(name="const", bufs=1))
    state = ctx.enter_context(tc.tile_pool(name="state", bufs=1))
    work = ctx.enter_context(tc.tile_pool(name="work", bufs=3))
    psum_pool = ctx.enter_context(tc.tile_pool(name="psum", bufs=1, space="PSUM"))

    cblob = const.tile([128, CONST_COLS], FP, tag="cblob")
    w_ih = cblob[:, OFF_WIH:OFF_WIH + 6 * G3]
    w_hh = cblob[:, OFF_WHH:OFF_WHH + 6 * G3]
    eye = cblob[:, OFF_EYE:OFF_EYE + 128]
    bias8 = cblob[0:8, OFF_B8:OFF_B8 + L * 128]
    sel8 = cblob[0:8, OFF_SEL:OFF_SEL + 512]

    ring = state.tile([128, LANES * RING], FP, tag="ring")
    xT = state.tile([128, 2 * 2 * W * B], FP, tag="xT")  # parity x (chunk, 64)
    x_raw = state.tile([W * B, 2 * H], FP, tag="x_raw")  # parity x 256
    out_stage = state.tile([W * B, 2 * H], FP, tag="out_stage")
    h0_raw = cblob[0:8, OFF_H0:OFF_H0 + L * H]
    hn_stage = state.tile([B, L * H], FP, tag="hn_stage")

    # 6 scan banks (layer-major, parity interleaved) + 2 scratch banks
    pscan = psum_pool.tile([128, 6 * 512], FP, tag="pscan")
    pscr = psum_pool.tile([128, 2 * 512], FP, tag="pscr")

    # ---- views ----
    # ring layout: col = (l*2+c)*(RING*B) + slot*B + b  -> matmul operands are
    # flat contiguous slices (BIR: matmul rhs must be single-free-dim)
    RV = ring[:, :].rearrange("p (l c t b) -> p l c t b", l=L, c=2, t=RING, b=B)
    PV = pscan[:, :].rearrange("p (l q s e) -> p l q s e", l=L, q=2, s=8, e=W * B)

    def ring_flat(l, kc, slot, n_slots=1):
        base = (l * 2 + kc) * (RING * B) + slot * B
        return ring[:, base:base + n_slots * B]

    def scr_ap(bank, p, f):
        # scratch transposes anchored at col 0 of each bank so successive
        # uses overlap -> Tile serializes (PE start zeroes the whole bank)
        return pscr[0:p, bank * 512: bank * 512 + f]

    # ---- init: ONE DMA for all constants (one semaphore for all the
    # early matmuls; walrus allows very few sync-waits per instruction) ----
    nc.sync.dma_start(cblob[:, :], consts_d[:, :])

    # h0 -> ring slots (8l-1) % RING, H-major via PE transpose
    for l in range(L):
        slot = (8 * l - 1) % RING
        for kc in range(2):
            scr = scr_ap(kc, 128, B)
            nc.tensor.transpose(
                scr, h0_raw[:, (l * 2 + kc) * 128:(l * 2 + kc + 1) * 128],
                eye[0:B, 0:B])
            nc.scalar.copy(ring_flat(l, kc, slot), scr)

    # ---- helpers ----
    def x_rowbase(c, it):
        # returns (dram_row_start expression, static?) rows are 64 per chunk
        if it is None:
            return c * W * B
        # c = c_const + 4*it in steady
        return it * (GRP_PER_ITER * W * B) + c * W * B

    def emit_xdma(cx, it=None):
        # DMA x rows for chunk cx into x_raw parity slot, then PE-transpose
        par = cx % 2
        if it is None:
            src = x_d[cx * W * B:(cx + 1) * W * B, :]
        else:
            base = it * (GRP_PER_ITER * W * B)
            src = x_d[bass.ds(base + cx * W * B, W * B), :]
        nc.sync.dma_start(x_raw[:, par * H:(par + 1) * H], src)
        for kc in range(2):
            scr = scr_ap(kc, 128, W * B)
            nc.tensor.transpose(
                scr,
                x_raw[:, par * H + kc * 128: par * H + (kc + 1) * 128],
                eye[0:W * B, 0:W * B])
            nc.scalar.copy(
                xT[:, par * 128 + kc * W * B: par * 128 + (kc + 1) * W * B], scr)

    def emit_bulk(lt, c):
        """Bias + input-projection matmuls filling PSUM window (lt, c)."""
        pp = (c + lt) % 2
        bank0 = (2 * lt + pp) * 512
        # seed the whole bank in one shot: bias8.T @ one-hot-selector
        # (start=True zeroes the entire 2KB bank, so exactly one start per
        #  (layer,parity) window)
        nc.tensor.matmul(
            pscan[:, bank0:bank0 + 512],
            bias8[:, lt * 128:(lt + 1) * 128],
            sel8[:, :],
            start=True, stop=False, skip_group_check=True)
        # gx matmuls (accumulate): m in 0..5, kc in 0..1
        for m in range(6):
            sub = _subreg(m)
            for kc in range(2):
                if lt == 0:
                    par = c % 2
                    rhs = xT[:, par * 128 + kc * W * B: par * 128 + (kc + 1) * W * B]
                else:
                    s0 = (8 * c + 8 * (lt - 1)) % RING
                    rhs = ring_flat(lt - 1, kc, s0, W)
                for j in range(4):
                    nc.tensor.matmul(
                        pscan[32 * j:32 * (j + 1), bank0 + sub * 64: bank0 + sub * 64 + W * B],
                        w_ih[:, (lt * 2 + kc) * G3 + m * 128 + j * 32:
                             (lt * 2 + kc) * G3 + m * 128 + (j + 1) * 32],
                        rhs,
                        start=False, stop=(kc == 1 and m >= 4),
                        skip_group_check=True,
                        tile_position=(0, 32 * j))

    def emit_outdma(co, it=None):
        """Transpose layer2 chunk co from ring -> out_stage -> DRAM."""
        par = co % 2
        s0 = (8 * co + 16) % RING
        for kc in range(2):
            scr = scr_ap(kc, W * B, 128)
            nc.tensor.transpose(scr, ring_flat(2, kc, s0, W), eye[:, :])
            nc.scalar.copy(
                out_stage[:, par * H + kc * 128: par * H + (kc + 1) * 128], scr)
        if it is None:
            dst = out_d[co * W * B:(co + 1) * W * B, :]
        else:
            base = it * (GRP_PER_ITER * W * B)
            dst = out_d[bass.ds(base + co * W * B, W * B), :]
        nc.sync.dma_start(dst, out_stage[:, par * H:(par + 1) * H])

    def emit_tick(cc, k):
        tau = cc * W + k
        slot_prev = (tau - 1) % RING
        slot_cur = tau % RING
        act = [l for l in range(L) if 0 <= cc - l < n_chunks]
        la0, nl = act[0], len(act)
        pp = cc % 2

        # --- recurrent matmuls: RZ tiles first, then HN ---
        def rec_mm(m_list, stop_last):
            for m in m_list:
                sub = m  # recurrent: rz -> subs 0..3, n -> HN subs 4,5
                for l in act:
                    bank0 = (2 * l + pp) * 512
                    for kc in range(2):
                        rhs = ring_flat(l, kc, slot_prev)
                        if COL_TILE_RECUR:
                            for j in range(4):
                                nc.tensor.matmul(
                                    pscan[32 * j:32 * (j + 1),
                                          bank0 + sub * 64 + k * B: bank0 + sub * 64 + (k + 1) * B],
                                    w_hh[:, (l * 2 + kc) * G3 + m * 128 + j * 32:
                                         (l * 2 + kc) * G3 + m * 128 + (j + 1) * 32],
                                    rhs, start=False, stop=(kc == 1),
                                    skip_group_check=True,
                                    tile_position=(0, 32 * j))
                        else:
                            nc.tensor.matmul(
                                pscan[:, bank0 + sub * 64 + k * B: bank0 + sub * 64 + (k + 1) * B],
                                w_hh[:, (l * 2 + kc) * G3 + m * 128:
                                     (l * 2 + kc) * G3 + (m + 1) * 128],
                                rhs, start=False, stop=(kc == 1),
                                skip_group_check=True)

        rec_mm([0, 1, 2, 3], False)

        # sigmoid(RZ) packed over active layers
        rz = work.tile([128, L * 4 * B], FP, tag="rz")
        ZV = rz[:, :].rearrange("p (l g b) -> p l g b", l=L, g=4, b=B)
        in_rz = PV[:, la0:la0 + nl, pp, 0:4, k * B:(k + 1) * B]
        nc.scalar.activation(ZV[:, la0:la0 + nl, :, :], in_rz, AF.Sigmoid)

        rec_mm([4, 5], True)

        r_ap = ZV[:, la0:la0 + nl, 0:2, :]
        z_ap = ZV[:, la0:la0 + nl, 2:4, :]
        hn_ap = PV[:, la0:la0 + nl, pp, 4:6, k * B:(k + 1) * B]
        xn_ap = PV[:, la0:la0 + nl, pp, 6:8, k * B:(k + 1) * B]

        def wv(tg):
            t = work.tile([128, LANES], FP, tag=tg)
            return t[:, :].rearrange("p (l c b) -> p l c b", l=L, c=2, b=B)

        t1 = wv("t1")
        t2 = wv("t2")
        nn = wv("nn")
        omz = wv("omz")
        uu = wv("uu")
        vv = wv("vv")
        sl = (slice(None), slice(la0, la0 + nl), slice(None), slice(None))

        nc.vector.tensor_tensor(t1[sl], hn_ap, r_ap, ALU.mult)
        nc.vector.tensor_tensor(t2[sl], t1[sl], xn_ap, ALU.add)
        nc.vector.tensor_scalar(omz[sl], z_ap, -1.0, 1.0, ALU.mult, ALU.add)
        ueng = nc.gpsimd if U_ON_GPSIMD else nc.vector
        ueng.tensor_tensor(uu[sl], RV[:, la0:la0 + nl, :, slot_prev, :], z_ap, ALU.mult)
        nc.scalar.activation(nn[sl], t2[sl], AF.Tanh)
        nc.vector.tensor_tensor(vv[sl], nn[sl], omz[sl], ALU.mult)
        nc.vector.tensor_tensor(RV[:, la0:la0 + nl, :, slot_cur, :], vv[sl], uu[sl],
                                ALU.add)

    def emit_group(cc, it=None, head_bulk=False):
        if head_bulk:
            # diagnostic mode: seed+bulk at head of consuming group
            for l in range(L):
                c = cc - l
                if 0 <= c < n_chunks:
                    emit_bulk(l, c)
        for k in range(W):
            emit_tick(cc, k)
        # tail: prefetch x, bulk projections, drain outputs
        if cc + 2 < n_chunks:
            emit_xdma(cc + 2, it)
        if not head_bulk:
            for l in range(L):
                c = cc - l + 1
                if 0 <= c < n_chunks:
                    emit_bulk(l, c)
        co = cc - 2
        if 0 <= co < n_chunks:
            emit_outdma(co, it)

    HEAD_BULK = bool(os.environ.get("GRU_HEAD_BULK"))
    # ---- prologue ----
    emit_xdma(0)
    emit_xdma(1)
    if not HEAD_BULK:
        emit_bulk(0, 0)
    emit_group(0, head_bulk=HEAD_BULK)
    tap0 = None
    if os.environ.get("GRU_DEBUG_RING"):
        tap0 = state.tile([128, 512], FP, tag="tap0")
        nc.vector.tensor_copy(tap0[:, :], pscan[:, 0:512])
    emit_group(1, head_bulk=HEAD_BULK)

    # ---- steady loop ----
    n_steady = n_groups - 6  # groups 2 .. n_groups-5
    assert n_steady % GRP_PER_ITER == 0
    n_iter = n_steady // GRP_PER_ITER
    if n_iter > 0:
        with tc.For_i(0, n_iter, 1) as it:
            for g in range(GRP_PER_ITER):
                emit_group(2 + g, it, head_bulk=HEAD_BULK)

    # ---- epilogue ----
    for cc in range(n_groups - 4, n_groups):
        emit_group(cc, head_bulk=HEAD_BULK)

    # h_n extraction
    for l in range(L):
        slot = (T - 1 + 8 * l) % RING
        for kc in range(2):
            scr = scr_ap(kc, B, 128)
            nc.tensor.transpose(scr, ring_flat(l, kc, slot), eye[:, :])
            nc.scalar.copy(
                hn_stage[0:B, (l * 2 + kc) * 128:(l * 2 + kc + 1) * 128], scr)
    nc.sync.dma_start(hn_d[:, :], hn_stage[:, :])

    dbg_d = globals().get("_DBG_D")
    if dbg_d is not None:
        nc.sync.dma_start(dbg_d[:, 0:LANES * RING], ring[:, :])
        psc = state.tile([128, 6 * 512], FP, tag="psc_dump")
        nc.vector.tensor_copy(psc[:, :], pscan[:, :])
        nc.sync.dma_start(dbg_d[:, LANES * RING:LANES * RING + 6 * 512], psc[:, :])
        nc.sync.dma_start(dbg_d[:, LANES * RING + 6 * 512:], tap0[:, :])

    ctx.close()


def _sel8():
    return np.kron(np.eye(8, dtype=np.float32),
                   np.ones((1, 64), dtype=np.float32)).astype(np.float32)


def _prep_host(x, h0, W_ih, W_hh, b_ih, b_hh):
    x = np.ascontiguousarray(np.asarray(x, dtype=np.float32))
    h0 = np.ascontiguousarray(np.asarray(h0, dtype=np.float32))
    W_ih = np.asarray(W_ih, dtype=np.float32)
    W_hh = np.asarray(W_hh, dtype=np.float32)
    b_ih = np.asarray(b_ih, dtype=np.float32)
    b_hh = np.asarray(b_hh, dtype=np.float32)

    T = x.shape[0]
    # Weight tiles: [l, kc, 128, 768] with kc the K (input-dim) chunk
    wih_t = np.ascontiguousarray(
        W_ih.transpose(0, 2, 1).reshape(L, 2, 128, G3)).reshape(L * 2 * 128, G3)
    whh_t = np.ascontiguousarray(
        W_hh.transpose(0, 2, 1).reshape(L, 2, 128, G3)).reshape(L * 2 * 128, G3)

    # bias_init[l, sub, 128]: sub 0..3 rz -> b_ih+b_hh ; 4,5 hn -> b_hh ; 6,7 xn -> b_ih
    bias_init = np.zeros((L, 8, 128), dtype=np.float32)
    comb = b_ih + b_hh
    for l in range(L):
        bias_init[l, 0:4] = comb[l, 0:512].reshape(4, 128)
        bias_init[l, 4:6] = b_hh[l, 512:768].reshape(2, 128)
        bias_init[l, 6:8] = b_ih[l, 512:768].reshape(2, 128)
    bias_init = np.ascontiguousarray(
        bias_init.transpose(1, 0, 2)).reshape(8, L * 128)

    eye = np.eye(128, dtype=np.float32)
    return T, x, h0, wih_t, whh_t, bias_init, eye


def _make_consts(wih_t, whh_t, bias_init, eye, h0_core):
    """h0_core: [L, B, H] for one core -> consts blob [128, CONST_COLS]."""
    c = np.zeros((128, CONST_COLS), np.float32)
    c[:, OFF_WIH:OFF_WIH + 6 * G3] = np.ascontiguousarray(
        wih_t.reshape(6, 128, G3).transpose(1, 0, 2)).reshape(128, 6 * G3)
    c[:, OFF_WHH:OFF_WHH + 6 * G3] = np.ascontiguousarray(
        whh_t.reshape(6, 128, G3).transpose(1, 0, 2)).reshape(128, 6 * G3)
    c[:, OFF_EYE:OFF_EYE + 128] = eye
    c[0:8, OFF_B8:OFF_B8 + L * 128] = bias_init
    c[0:8, OFF_SEL:OFF_SEL + 512] = _sel8()
    c[0:8, OFF_H0:OFF_H0 + L * H] = np.ascontiguousarray(
        h0_core.transpose(1, 0, 2)).reshape(B, L * H)
    return c


def _run_spmd_bench(nc, in_maps, reps):
    """Replicates bass2jax.run_bass_via_pjrt multi-core path without output
    donation so the jitted executable can be re-run; returns (results,
    per-iter seconds) with inputs resident on device."""
    import time

    import jax
    from jax.sharding import Mesh, PartitionSpec, NamedSharding
    from jax.experimental.shard_map import shard_map

    from concourse import bass2jax as b2j
    import concourse.mybir as mb

    b2j.install_neuronx_cc_hook()
    n_cores = len(in_maps)
    partition_name = nc.partition_id_tensor.name if nc.partition_id_tensor else None

    in_names, out_names, out_avals, zero_outs = [], [], [], []
    for alloc in nc.m.functions[0].allocations:
        if not isinstance(alloc, mb.MemoryLocationSet):
            continue
        name = alloc.memorylocations[0].name
        if alloc.kind == "ExternalInput":
            if name != partition_name:
                in_names.append(name)
        elif alloc.kind == "ExternalOutput":
            out_names.append(name)
            shape = tuple(alloc.tensor_shape)
            dtype = mb.dt.np(alloc.dtype)
            out_avals.append(jax.core.ShapedArray(shape, dtype))
            zero_outs.append(np.zeros(shape, dtype))
    n_params = len(in_names)
    in_names.extend(out_names)
    if partition_name is not None:
        in_names.append(partition_name)

    def _body(*args):
        operands = list(args)
        if partition_name is not None:
            operands.append(b2j.partition_id_tensor())
        outs = b2j._bass_exec_p.bind(
            *operands,
            out_avals=tuple(out_avals),
            in_names=tuple(in_names),
            out_names=tuple(out_names),
            lowering_input_output_aliases=(),
            sim_require_finite=True,
            sim_require_nnan=True,
            nc=nc,
        )
        return tuple(outs)

    devices = jax.devices()[:n_cores]
    mesh = Mesh(np.asarray(devices), ("core",))
    n_outs = len(out_avals)
    in_specs = (PartitionSpec("core"),) * (n_params + n_outs)
    out_specs = (PartitionSpec("core"),) * n_outs
    sharded = jax.jit(
        shard_map(_body, mesh=mesh, in_specs=in_specs, out_specs=out_specs,
                  check_rep=False),
        keep_unused=True,
    )
    sh = NamedSharding(mesh, PartitionSpec("core"))
    concat_in = [
        jax.device_put(
            np.concatenate([np.asarray(in_maps[c][in_names[i]]) for c in range(n_cores)], axis=0),
            sh)
        for i in range(n_params)
    ]
    concat_zeros = [
        jax.device_put(np.zeros((n_cores * z.shape[0], *z.shape[1:]), z.dtype), sh)
        for z in zero_outs
    ]
    out_arrs = sharded(*concat_in, *concat_zeros)
    jax.block_until_ready(out_arrs)
    t0 = time.perf_counter()
    for _ in range(reps):
        last = sharded(*concat_in, *concat_zeros)
    jax.block_until_ready(last)
    dt = (time.perf_counter() - t0) / reps

    results = [
        {name: np.asarray(out_arrs[i]).reshape(n_cores, *out_avals[i].shape)[c]
         for i, name in enumerate(out_names)}
        for c in range(n_cores)
    ]
    return results, dt


def kernel(x, h0, W_ih, W_hh, b_ih, b_hh):
    global LAST_EXEC_NS, LAST_RESULTS
    T, x, h0, wih_t, whh_t, bias_init, eye = _prep_host(x, h0, W_ih, W_hh, b_ih, b_hh)

    nc = build_program(T)

    in_maps = []
    for ci in range(N_CORES):
        bsl = slice(ci * B, (ci + 1) * B)
        in_maps.append({
            "x": np.ascontiguousarray(x[:, bsl, :]).reshape(T * B, H),
            "consts": _make_consts(wih_t, whh_t, bias_init, eye, h0[:, bsl, :]),
        })

    reps = int(os.environ.get("GRU_BENCH_REPS", "0"))
    if reps > 0:
        results, dt = _run_spmd_bench(nc, in_maps, reps)
        LAST_EXEC_NS = int(dt * 1e9)
    else:
        res = run_bass_kernel_spmd(nc, in_maps, list(range(N_CORES)))
        LAST_EXEC_NS = getattr(res, "exec_time_ns", None)
        LAST_RESULTS = res
        results = res.results

    T_ = T
    out = np.empty((T_, N_CORES * B, H), dtype=np.float32)
    h_n = np.empty((L, N_CORES * B, H), dtype=np.float32)
    for ci in range(N_CORES):
        bsl = slice(ci * B, (ci + 1) * B)
        out[:, bsl, :] = results[ci]["out"].reshape(T_, B, H)
        h_n[:, bsl, :] = results[ci]["h_n"].reshape(B, L, H).transpose(1, 0, 2)
    return out, h_n


# revision 31
# speedup vs baseline: 1.8354x; 1.8354x over previous
"""Trainium2 Bass kernel for a 3-layer GRU (T=2048, B=64, IN=H=256).

Strategy (8 NeuronCores, data-parallel over batch, B_local=8 per core):
  - Everything lives H-major on-chip: hidden vectors are [H(part), batch(free)].
  - Per layer a PSUM window of W=8 timesteps holds the gate pre-activations:
      RZ  = x@Wih_rz + b_ih_rz + b_hh_rz  (+= Whh_rz @ h per step)
      HN  = b_hh_n                        (+= Whh_n  @ h per step)
      XN  = x@Wih_n  + b_ih_n             (bulk only)
    The bulk input projection is a real matmul accumulated straight into
    PSUM (start=True bias matmul with a ones-vector rhs seeds the biases).
  - The sequential scan runs layer-pipelined: at tick t, layer0 processes
    step t, layer1 step t-8, layer2 step t-16. Gate elementwise ops for all
    active layers are packed into single wide instructions ([128, 48]-ish).
  - h state lives in a 32-slot SBUF ring [128, 48*32]; slot = tick % 32.
  - Recurrent matmuls are weight-stationary, column-tiled 4x (tile_position)
    so four 32-col LDWEIGHTS/MATMULs proceed concurrently.
  - Steady state runs under tc.For_i with a 4-group (32-tick) body so all
    ring/PSUM offsets are static; only DRAM DMA offsets use the loop var.
"""

import os
import numpy as np

try:
    import concourse.bass as bass
except ImportError:  # pragma: no cover
    import sys

    for p in ("/opt/trn_rl_repo", "/root/.axon_site/_ro/trn_rl_repo"):
        if os.path.isdir(p) and p not in sys.path:
            sys.path.insert(0, p)
    import concourse.bass as bass

import concourse.mybir as mybir
import concourse.tile as tile
from concourse.bass_utils import run_bass_kernel_spmd

FP = mybir.dt.float32
AF = mybir.ActivationFunctionType
ALU = mybir.AluOpType

N_CORES = 8
T_FULL = 2048
B = 8  # batch per core
H = 256
G3 = 768
L = 3
W = 8  # steps per window (= chunk)
RING = 32  # ring slots (ticks); multiple of W, >= 4 windows
GRP_PER_ITER = 4  # groups per For_i iteration (RING // W)
LANES = L * 2 * B  # 48 cols per ring slot: (layer, h-chunk, batch)

# consts blob column offsets: [wih | whh | eye | bias8 | sel8 | h0]
OFF_WIH = 0
OFF_WHH = 6 * G3
OFF_EYE = 12 * G3
OFF_B8 = 12 * G3 + 128
OFF_SEL = OFF_B8 + L * 128
OFF_H0 = OFF_SEL + 512
CONST_COLS = OFF_H0 + L * H

# Knobs
U_ON_GPSIMD = False  # z*h_prev on gpsimd instead of vector
COL_TILE_RECUR = True  # 4x column-tiled recurrent matmuls

LAST_EXEC_NS = None
LAST_RESULTS = None


def _subreg(m):
    # Wih/Whh output M-tile m (gate rows 128m..128m+128) -> PSUM subregion.
    # Subregions: 0..3 = RZ, 4..5 = HN (recurrent n), 6..7 = XN (bulk n).
    return m if m < 4 else m + 2


def build_program(T, legalize=True):
    assert T % (W * GRP_PER_ITER) == 0
    n_chunks = T // W
    n_groups = n_chunks + L - 1

    nc = bass.Bass()

    x_d = nc.declare_dram_parameter("x", [T * B, H], FP, isOutput=False)
    consts_d = nc.declare_dram_parameter("consts", [128, CONST_COLS], FP,
                                         isOutput=False)
    out_d = nc.declare_dram_parameter("out", [T * B, H], FP, isOutput=True)
    hn_d = nc.declare_dram_parameter("h_n", [B, L * H], FP, isOutput=True)
    dbg_d = None
    if os.environ.get("GRU_DEBUG_RING"):
        dbg_d = nc.declare_dram_parameter("dbg", [128, LANES * RING + 7 * 512], FP,
                                          isOutput=True)
    globals()["_DBG_D"] = dbg_d

    with tile.TileContext(nc) as tc:
        _build_tiles(tc, nc, T, n_chunks, n_groups,
                     x_d, consts_d, out_d, hn_d)
    if legalize:
        _legalize_waits(nc)
    return nc


def _legalize_waits(nc):
    """This walrus build allows only ONE embedded sync-wait per compute
    instruction (LDWEIGHTS/ACT structs overflow at 2). Hoist extra waits
    onto an inserted same-engine Drain, which supports many waits."""
    CAP_DRAIN = 1
    n_fix = 0
    for f in nc.m.functions:
        for b in f.blocks:
            out = []
            changed = False
            for inst in b.instructions:
                si = getattr(inst, 'sync_info', None)
                waits = list(si.on_wait) if si is not None and si.on_wait else []
                opname = type(inst).__name__
                cap = CAP_DRAIN if opname == "InstDrain" else 1
                if len(waits) > cap:
                    extra = waits[:-cap]
                    si.on_wait = waits[-cap:]
                    for j in range(0, len(extra), CAP_DRAIN):
                        d = mybir.InstDrain(name=f"I-wfix-{n_fix}", ins=[], outs=[])
                        n_fix += 1
                        d.engine = inst.engine
                        d.sync_info = mybir.SyncInfo(
                            on_wait=extra[j:j + CAP_DRAIN], on_update=[])
                        out.append(d)
                    changed = True
                out.append(inst)
            if changed:
                b.instructions = out
    return n_fix


def _build_tiles(tc, nc, T, n_chunks, n_groups,
                 x_d, consts_d, out_d, hn_d):
    from contextlib import ExitStack

    ctx = ExitStack()
    const = ctx.enter_context(tc.tile_pool(name="const", bufs=1))
    state = ctx.enter_context(tc.tile_pool(name="state", bufs=1))
    work = ctx.enter_context(tc.tile_pool(name="work", bufs=8))
    psum_pool = ctx.enter_context(tc.tile_pool(name="psum", bufs=1, space="PSUM"))

    cblob = const.tile([128, CONST_COLS], FP, tag="cblob")
    w_ih = cblob[:, OFF_WIH:OFF_WIH + 6 * G3]
    w_hh = cblob[:, OFF_WHH:OFF_WHH + 6 * G3]
    eye = cblob[:, OFF_EYE:OFF_EYE + 128]
    bias8 = cblob[0:8, OFF_B8:OFF_B8 + L * 128]
    sel8 = cblob[0:8, OFF_SEL:OFF_SEL + 512]

    ring = state.tile([128, LANES * RING], FP, tag="ring")
    xT = state.tile([128, 2 * 2 * W * B], FP, tag="xT")  # parity x (chunk, 64)
    x_raw = state.tile([W * B, 2 * H], FP, tag="x_raw")  # parity x 256
    out_stage = state.tile([W * B, 2 * H], FP, tag="out_stage")
    h0_raw = cblob[0:8, OFF_H0:OFF_H0 + L * H]
    hn_stage = state.tile([B, L * H], FP, tag="hn_stage")

    # 6 scan banks (layer-major, parity interleaved) + 2 scratch banks
    pscan = psum_pool.tile([128, 6 * 512], FP, tag="pscan")
    pscr = psum_pool.tile([128, 2 * 512], FP, tag="pscr")

    # ---- views ----
    # ring layout: col = (l*2+c)*(RING*B) + slot*B + b  -> matmul operands are
    # flat contiguous slices (BIR: matmul rhs must be single-free-dim)
    RV = ring[:, :].rearrange("p (l c t b) -> p l c t b", l=L, c=2, t=RING, b=B)
    PV = pscan[:, :].rearrange("p (l q s e) -> p l q s e", l=L, q=2, s=8, e=W * B)

    def ring_flat(l, kc, slot, n_slots=1):
        base = (l * 2 + kc) * (RING * B) + slot * B
        return ring[:, base:base + n_slots * B]

    def scr_ap(bank, p, f):
        # scratch transposes anchored at col 0 of each bank so successive
        # uses overlap -> Tile serializes (PE start zeroes the whole bank)
        return pscr[0:p, bank * 512: bank * 512 + f]

    # ---- init: ONE DMA for all constants (one semaphore for all the
    # early matmuls; walrus allows very few sync-waits per instruction) ----
    nc.sync.dma_start(cblob[:, :], consts_d[:, :])

    # h0 -> ring slots (8l-1) % RING, H-major via PE transpose
    for l in range(L):
        slot = (8 * l - 1) % RING
        for kc in range(2):
            scr = scr_ap(kc, 128, B)
            nc.tensor.transpose(
                scr, h0_raw[:, (l * 2 + kc) * 128:(l * 2 + kc + 1) * 128],
                eye[0:B, 0:B])
            nc.scalar.copy(ring_flat(l, kc, slot), scr)

    # ---- helpers ----
    def x_rowbase(c, it):
        # returns (dram_row_start expression, static?) rows are 64 per chunk
        if it is None:
            return c * W * B
        # c = c_const + 4*it in steady
        return it * (GRP_PER_ITER * W * B) + c * W * B

    def emit_xdma(cx, it=None):
        # DMA x rows for chunk cx into x_raw parity slot, then PE-transpose
        par = cx % 2
        if it is None:
            src = x_d[cx * W * B:(cx + 1) * W * B, :]
        else:
            base = it * (GRP_PER_ITER * W * B)
            src = x_d[bass.ds(base + cx * W * B, W * B), :]
        nc.sync.dma_start(x_raw[:, par * H:(par + 1) * H], src)
        for kc in range(2):
            scr = scr_ap(kc, 128, W * B)
            nc.tensor.transpose(
                scr,
                x_raw[:, par * H + kc * 128: par * H + (kc + 1) * 128],
                eye[0:W * B, 0:W * B])
            nc.scalar.copy(
                xT[:, par * 128 + kc * W * B: par * 128 + (kc + 1) * W * B], scr)

    def emit_bulk(lt, c):
        """Bias + input-projection matmuls filling PSUM window (lt, c)."""
        pp = (c + lt) % 2
        bank0 = (2 * lt + pp) * 512
        # seed the whole bank in one shot: bias8.T @ one-hot-selector
        # (start=True zeroes the entire 2KB bank, so exactly one start per
        #  (layer,parity) window)
        nc.tensor.matmul(
            pscan[:, bank0:bank0 + 512],
            bias8[:, lt * 128:(lt + 1) * 128],
            sel8[:, :],
            start=True, stop=False, skip_group_check=True)
        # gx matmuls (accumulate): m in 0..5, kc in 0..1
        for m in range(6):
            sub = _subreg(m)
            for kc in range(2):
                if lt == 0:
                    par = c % 2
                    rhs = xT[:, par * 128 + kc * W * B: par * 128 + (kc + 1) * W * B]
                else:
                    s0 = (8 * c + 8 * (lt - 1)) % RING
                    rhs = ring_flat(lt - 1, kc, s0, W)
                for j in range(4):
                    nc.tensor.matmul(
                        pscan[32 * j:32 * (j + 1), bank0 + sub * 64: bank0 + sub * 64 + W * B],
                        w_ih[:, (lt * 2 + kc) * G3 + m * 128 + j * 32:
                             (lt * 2 + kc) * G3 + m * 128 + (j + 1) * 32],
                        rhs,
                        start=False, stop=(kc == 1 and m >= 4),
                        skip_group_check=True,
                        tile_position=(0, 32 * j))

    def emit_outdma(co, it=None):
        """Transpose layer2 chunk co from ring -> out_stage -> DRAM."""
        par = co % 2
        s0 = (8 * co + 16) % RING
        for kc in range(2):
            scr = scr_ap(kc, W * B, 128)
            nc.tensor.transpose(scr, ring_flat(2, kc, s0, W), eye[:, :])
            nc.scalar.copy(
                out_stage[:, par * H + kc * 128: par * H + (kc + 1) * 128], scr)
        if it is None:
            dst = out_d[co * W * B:(co + 1) * W * B, :]
        else:
            base = it * (GRP_PER_ITER * W * B)
            dst = out_d[bass.ds(base + co * W * B, W * B), :]
        nc.sync.dma_start(dst, out_stage[:, par * H:(par + 1) * H])

    def emit_tick(cc, k):
        tau = cc * W + k
        slot_prev = (tau - 1) % RING
        slot_cur = tau % RING
        act = [l for l in range(L) if 0 <= cc - l < n_chunks]
        la0, nl = act[0], len(act)
        pp = cc % 2

        # --- recurrent matmuls: RZ tiles first, then HN ---
        def rec_mm(m_list, stop_last):
            for m in m_list:
                sub = m  # recurrent: rz -> subs 0..3, n -> HN subs 4,5
                for l in act:
                    bank0 = (2 * l + pp) * 512
                    for kc in range(2):
                        rhs = ring_flat(l, kc, slot_prev)
                        if COL_TILE_RECUR:
                            for j in range(4):
                                nc.tensor.matmul(
                                    pscan[32 * j:32 * (j + 1),
                                          bank0 + sub * 64 + k * B: bank0 + sub * 64 + (k + 1) * B],
                                    w_hh[:, (l * 2 + kc) * G3 + m * 128 + j * 32:
                                         (l * 2 + kc) * G3 + m * 128 + (j + 1) * 32],
                                    rhs, start=False, stop=(kc == 1),
                                    skip_group_check=True,
                                    tile_position=(0, 32 * j))
                        else:
                            nc.tensor.matmul(
                                pscan[:, bank0 + sub * 64 + k * B: bank0 + sub * 64 + (k + 1) * B],
                                w_hh[:, (l * 2 + kc) * G3 + m * 128:
                                     (l * 2 + kc) * G3 + (m + 1) * 128],
                                rhs, start=False, stop=(kc == 1),
                                skip_group_check=True)

        rec_mm([0, 1, 2, 3], False)

        # sigmoid(RZ) packed over active layers
        rz = work.tile([128, L * 4 * B], FP, tag="rz")
        ZV = rz[:, :].rearrange("p (l g b) -> p l g b", l=L, g=4, b=B)
        in_rz = PV[:, la0:la0 + nl, pp, 0:4, k * B:(k + 1) * B]
        nc.scalar.activation(ZV[:, la0:la0 + nl, :, :], in_rz, AF.Sigmoid)

        rec_mm([4, 5], True)

        r_ap = ZV[:, la0:la0 + nl, 0:2, :]
        z_ap = ZV[:, la0:la0 + nl, 2:4, :]
        hn_ap = PV[:, la0:la0 + nl, pp, 4:6, k * B:(k + 1) * B]
        xn_ap = PV[:, la0:la0 + nl, pp, 6:8, k * B:(k + 1) * B]

        def wv(tg):
            t = work.tile([128, LANES], FP, tag=tg)
            return t[:, :].rearrange("p (l c b) -> p l c b", l=L, c=2, b=B)

        t1 = wv("t1")
        t2 = wv("t2")
        nn = wv("nn")
        omz = wv("omz")
        uu = wv("uu")
        vv = wv("vv")
        sl = (slice(None), slice(la0, la0 + nl), slice(None), slice(None))

        nc.vector.tensor_tensor(t1[sl], hn_ap, r_ap, ALU.mult)
        nc.vector.tensor_tensor(t2[sl], t1[sl], xn_ap, ALU.add)
        nc.vector.tensor_scalar(omz[sl], z_ap, -1.0, 1.0, ALU.mult, ALU.add)
        ueng = nc.gpsimd if U_ON_GPSIMD else nc.vector
        ueng.tensor_tensor(uu[sl], RV[:, la0:la0 + nl, :, slot_prev, :], z_ap, ALU.mult)
        nc.scalar.activation(nn[sl], t2[sl], AF.Tanh)
        nc.vector.tensor_tensor(vv[sl], nn[sl], omz[sl], ALU.mult)
        nc.vector.tensor_tensor(RV[:, la0:la0 + nl, :, slot_cur, :], vv[sl], uu[sl],
                                ALU.add)

    def emit_group(cc, it=None, head_bulk=False):
        if head_bulk:
            # diagnostic mode: seed+bulk at head of consuming group
            for l in range(L):
                c = cc - l
                if 0 <= c < n_chunks:
                    emit_bulk(l, c)
        for k in range(W):
            emit_tick(cc, k)
        # tail: prefetch x, bulk projections, drain outputs
        if cc + 2 < n_chunks:
            emit_xdma(cc + 2, it)
        if not head_bulk:
            for l in range(L):
                c = cc - l + 1
                if 0 <= c < n_chunks:
                    emit_bulk(l, c)
        co = cc - 2
        if 0 <= co < n_chunks:
            emit_outdma(co, it)

    HEAD_BULK = bool(os.environ.get("GRU_HEAD_BULK"))
    # ---- prologue ----
    emit_xdma(0)
    emit_xdma(1)
    if not HEAD_BULK:
        emit_bulk(0, 0)
    emit_group(0, head_bulk=HEAD_BULK)
    tap0 = None
    if os.environ.get("GRU_DEBUG_RING"):
        tap0 = state.tile([128, 512], FP, tag="tap0")
        nc.vector.tensor_copy(tap0[:, :], pscan[:, 0:512])
    emit_group(1, head_bulk=HEAD_BULK)

    # ---- steady loop ----
    n_steady = n_groups - 6  # groups 2 .. n_groups-5
    assert n_steady % GRP_PER_ITER == 0
    n_iter = n_steady // GRP_PER_ITER
    if n_iter > 0:
        with tc.For_i(0, n_iter, 1) as it:
            for g in range(GRP_PER_ITER):
                emit_group(2 + g, it, head_bulk=HEAD_BULK)

    # ---- epilogue ----
    for cc in range(n_groups - 4, n_groups):
        emit_group(cc, head_bulk=HEAD_BULK)

    # h_n extraction
    for l in range(L):
        slot = (T - 1 + 8 * l) % RING
        for kc in range(2):
            scr = scr_ap(kc, B, 128)
            nc.tensor.transpose(scr, ring_flat(l, kc, slot), eye[:, :])
            nc.scalar.copy(
                hn_stage[0:B, (l * 2 + kc) * 128:(l * 2 + kc + 1) * 128], scr)
    nc.sync.dma_start(hn_d[:, :], hn_stage[:, :])

    dbg_d = globals().get("_DBG_D")
    if dbg_d is not None:
        nc.sync.dma_start(dbg_d[:, 0:LANES * RING], ring[:, :])
        psc = state.tile([128, 6 * 512], FP, tag="psc_dump")
        nc.vector.tensor_copy(psc[:, :], pscan[:, :])
        nc.sync.dma_start(dbg_d[:, LANES * RING:LANES * RING + 6 * 512], psc[:, :])
        nc.sync.dma_start(dbg_d[:, LANES * RING + 6 * 512:], tap0[:, :])

    ctx.close()


def _sel8():
    return np.kron(np.eye(8, dtype=np.float32),
                   np.ones((1, 64), dtype=np.float32)).astype(np.float32)


def _prep_host(x, h0, W_ih, W_hh, b_ih, b_hh):
    x = np.ascontiguousarray(np.asarray(x, dtype=np.float32))
    h0 = np.ascontiguousarray(np.asarray(h0, dtype=np.float32))
    W_ih = np.asarray(W_ih, dtype=np.float32)
    W_hh = np.asarray(W_hh, dtype=np.float32)
    b_ih = np.asarray(b_ih, dtype=np.float32)
    b_hh = np.asarray(b_hh, dtype=np.float32)

    T = x.shape[0]
    # Weight tiles: [l, kc, 128, 768] with kc the K (input-dim) chunk
    wih_t = np.ascontiguousarray(
        W_ih.transpose(0, 2, 1).reshape(L, 2, 128, G3)).reshape(L * 2 * 128, G3)
    whh_t = np.ascontiguousarray(
        W_hh.transpose(0, 2, 1).reshape(L, 2, 128, G3)).reshape(L * 2 * 128, G3)

    # bias_init[l, sub, 128]: sub 0..3 rz -> b_ih+b_hh ; 4,5 hn -> b_hh ; 6,7 xn -> b_ih
    bias_init = np.zeros((L, 8, 128), dtype=np.float32)
    comb = b_ih + b_hh
    for l in range(L):
        bias_init[l, 0:4] = comb[l, 0:512].reshape(4, 128)
        bias_init[l, 4:6] = b_hh[l, 512:768].reshape(2, 128)
        bias_init[l, 6:8] = b_ih[l, 512:768].reshape(2, 128)
    bias_init = np.ascontiguousarray(
        bias_init.transpose(1, 0, 2)).reshape(8, L * 128)

    eye = np.eye(128, dtype=np.float32)
    return T, x, h0, wih_t, whh_t, bias_init, eye


def _make_consts(wih_t, whh_t, bias_init, eye, h0_core):
    """h0_core: [L, B, H] for one core -> consts blob [128, CONST_COLS]."""
    c = np.zeros((128, CONST_COLS), np.float32)
    c[:, OFF_WIH:OFF_WIH + 6 * G3] = np.ascontiguousarray(
        wih_t.reshape(6, 128, G3).transpose(1, 0, 2)).reshape(128, 6 * G3)
    c[:, OFF_WHH:OFF_WHH + 6 * G3] = np.ascontiguousarray(
        whh_t.reshape(6, 128, G3).transpose(1, 0, 2)).reshape(128, 6 * G3)
    c[:, OFF_EYE:OFF_EYE + 128] = eye
    c[0:8, OFF_B8:OFF_B8 + L * 128] = bias_init
    c[0:8, OFF_SEL:OFF_SEL + 512] = _sel8()
    c[0:8, OFF_H0:OFF_H0 + L * H] = np.ascontiguousarray(
        h0_core.transpose(1, 0, 2)).reshape(B, L * H)
    return c


def _run_spmd_bench(nc, in_maps, reps):
    """Replicates bass2jax.run_bass_via_pjrt multi-core path without output
    donation so the jitted executable can be re-run; returns (results,
    per-iter seconds) with inputs resident on device."""
    import time

    import jax
    from jax.sharding import Mesh, PartitionSpec, NamedSharding
    from jax.experimental.shard_map import shard_map

    from concourse import bass2jax as b2j
    import concourse.mybir as mb

    b2j.install_neuronx_cc_hook()
    n_cores = len(in_maps)
    partition_name = nc.partition_id_tensor.name if nc.partition_id_tensor else None

    in_names, out_names, out_avals, zero_outs = [], [], [], []
    for alloc in nc.m.functions[0].allocations:
        if not isinstance(alloc, mb.MemoryLocationSet):
            continue
        name = alloc.memorylocations[0].name
        if alloc.kind == "ExternalInput":
            if name != partition_name:
                in_names.append(name)
        elif alloc.kind == "ExternalOutput":
            out_names.append(name)
            shape = tuple(alloc.tensor_shape)
            dtype = mb.dt.np(alloc.dtype)
            out_avals.append(jax.core.ShapedArray(shape, dtype))
            zero_outs.append(np.zeros(shape, dtype))
    n_params = len(in_names)
    in_names.extend(out_names)
    if partition_name is not None:
        in_names.append(partition_name)

    def _body(*args):
        operands = list(args)
        if partition_name is not None:
            operands.append(b2j.partition_id_tensor())
        outs = b2j._bass_exec_p.bind(
            *operands,
            out_avals=tuple(out_avals),
            in_names=tuple(in_names),
            out_names=tuple(out_names),
            lowering_input_output_aliases=(),
            sim_require_finite=True,
            sim_require_nnan=True,
            nc=nc,
        )
        return tuple(outs)

    devices = jax.devices()[:n_cores]
    mesh = Mesh(np.asarray(devices), ("core",))
    n_outs = len(out_avals)
    in_specs = (PartitionSpec("core"),) * (n_params + n_outs)
    out_specs = (PartitionSpec("core"),) * n_outs
    sharded = jax.jit(
        shard_map(_body, mesh=mesh, in_specs=in_specs, out_specs=out_specs,
                  check_rep=False),
        keep_unused=True,
    )
    sh = NamedSharding(mesh, PartitionSpec("core"))
    concat_in = [
        jax.device_put(
            np.concatenate([np.asarray(in_maps[c][in_names[i]]) for c in range(n_cores)], axis=0),
            sh)
        for i in range(n_params)
    ]
    concat_zeros = [
        jax.device_put(np.zeros((n_cores * z.shape[0], *z.shape[1:]), z.dtype), sh)
        for z in zero_outs
    ]
    out_arrs = sharded(*concat_in, *concat_zeros)
    jax.block_until_ready(out_arrs)
    t0 = time.perf_counter()
    for _ in range(reps):
        last = sharded(*concat_in, *concat_zeros)
    jax.block_until_ready(last)
    dt = (time.perf_counter() - t0) / reps

    results = [
        {name: np.asarray(out_arrs[i]).reshape(n_cores, *out_avals[i].shape)[c]
         for i, name in enumerate(out_names)}
        for c in range(n_cores)
    ]
    return results, dt


def kernel(x, h0, W_ih, W_hh, b_ih, b_hh):
    global LAST_EXEC_NS, LAST_RESULTS
    T, x, h0, wih_t, whh_t, bias_init, eye = _prep_host(x, h0, W_ih, W_hh, b_ih, b_hh)

    nc = build_program(T)

    in_maps = []
    for ci in range(N_CORES):
        bsl = slice(ci * B, (ci + 1) * B)
        in_maps.append({
            "x": np.ascontiguousarray(x[:, bsl, :]).reshape(T * B, H),
            "consts": _make_consts(wih_t, whh_t, bias_init, eye, h0[:, bsl, :]),
        })

    reps = int(os.environ.get("GRU_BENCH_REPS", "0"))
    if reps > 0:
        results, dt = _run_spmd_bench(nc, in_maps, reps)
        LAST_EXEC_NS = int(dt * 1e9)
    else:
        res = run_bass_kernel_spmd(nc, in_maps, list(range(N_CORES)))
        LAST_EXEC_NS = getattr(res, "exec_time_ns", None)
        LAST_RESULTS = res
        results = res.results

    T_ = T
    out = np.empty((T_, N_CORES * B, H), dtype=np.float32)
    h_n = np.empty((L, N_CORES * B, H), dtype=np.float32)
    for ci in range(N_CORES):
        bsl = slice(ci * B, (ci + 1) * B)
        out[:, bsl, :] = results[ci]["out"].reshape(T_, B, H)
        h_n[:, bsl, :] = results[ci]["h_n"].reshape(B, L, H).transpose(1, 0, 2)
    return out, h_n
